# revision 1
# baseline (speedup 1.0000x reference)
"""GRU-D layer kernel v2: sequence-parallel segments with warmup.

Shapes: x [256, 512, 128], h_decay [256, 512], H=256. 8 cores, batch 32/core.

Sequence split: S=8 segments of L=64 steps, W=8 warmup steps from h=0
(contraction (1-z)*d makes truncation error ~1e-4; bf16 rounding dominates).
Per core: V=256 virtual sequences (32 batch x 8 segments), M=72 macro-steps.

Layouts (per core):
  h state      [128 p, 512]  bf16, col = mc*256 + 32*s + b  (h = 128*mc + p)
  psum_zr      [128, 1024] f32, col = (gate*2+mc)*256 + v, gate r=0 z=1
  psum_h       [128, 512]  f32
  proj DRAM    [72, 128, 1536] bf16, col = gc*256 + v, gc=(gate)*2+mc,
               gate order r,z,h; bias included (zero for padded steps)
  decb         [9, 128, 8*512] bf16 (d broadcast, col as h state, per step)
  out          [8, 128, 8*512] bf16 groups (i>=8), host converts to f32

Prologue: proj = x @ [Wr|Wz|Wh] (+bias via ACT) computed TRANSPOSED
(lhsT=W chunk, rhs=x supertile [128 d, 512 bt]) straight into DRAM scratch.

Recurrence per macro-step (no identity folds, psum decoupled from ACT):
  PE : 4 mm r-gate, 4 mm z-gate (on hdec), 4 mm h-gate (on rh)
  DVE: m2_r = psum_r + proj_r; m2_z; rh = r*hdec; a = (z-1)*hdec;
       mh = psum_h + proj_h; bb = z*ht; hnew = bb - a; hdec' = d*hnew
  ACT: r = sig(m2_r); z = sig(m2_z); ht = tanh(mh)
"""

import numpy as np

B, T, D, H = 256, 512, 128, 256
NCORES = 8
BS = B // NCORES          # 32 batch per core
S = 8                     # segments
L = T // S                # 64
W = 8                     # warmup steps
M = L + W                 # 72 macro-steps
V = BS * S                # 256 virtual seqs per core
GS = 8                    # steps per dec/out group
G = M // GS               # 9 groups (group 0 = warmup, no output)
NSUP = T // 16            # 32 prologue supertiles

TRACE = False
LAST_EXEC_NS = None

_NC_CACHE = {}


def _build(variant=()):
    vset = set(variant)
    import concourse.bass as bass
    import concourse.mybir as mybir
    from concourse.tile import TileContext

    f32 = mybir.dt.float32
    bf16 = mybir.dt.bfloat16
    SIG = mybir.ActivationFunctionType.Sigmoid
    TANH = mybir.ActivationFunctionType.Tanh
    IDN = mybir.ActivationFunctionType.Identity
    MUL = mybir.AluOpType.mult
    SUB = mybir.AluOpType.subtract

    nc = bass.Bass()
    xT_d = nc.dram_tensor("xT", [NSUP, 128, 512], f32, kind="ExternalInput")
    W_d = nc.dram_tensor("Wc", [128, 768], f32, kind="ExternalInput")
    b3T_d = nc.dram_tensor("b3T", [128, 8], f32, kind="ExternalInput")
    uzr_d = nc.dram_tensor("Uzr", [8, 128, 128], bf16, kind="ExternalInput")
    uh_d = nc.dram_tensor("Uh4", [4, 128, 128], bf16, kind="ExternalInput")
    i128_d = nc.dram_tensor("I128", [128, 128], bf16, kind="ExternalInput")
    decb_d = nc.dram_tensor("decb", [G, 128, GS * 512], bf16,
                            kind="ExternalInput")
    outG_d = nc.dram_tensor("outG", [G - 1, 128, GS * 512], bf16,
                            kind="ExternalOutput")

    with TileContext(nc) as tc:
        # DRAM scratch for projections: one tile per GS-step group so the
        # recurrence's group g load only depends on its own prologue writers.
        # r/z parts are bf16 (merged on DVE); the h part is f32 in psum
        # order and DMA-preloaded straight into the h psum bank each step.
        projs, projhs, _free_projs = [], [], []
        for g_ in range(G):
            t_, f_ = tc.tile([GS, 128, 1024], bf16, space="DRAM",
                             name=f"projscratch{g_}")
            projs.append(t_)
            _free_projs.append(f_)
            th_, fh_ = tc.tile([GS, 128, 512], bf16, space="DRAM",
                               name=f"projhscratch{g_}")
            projhs.append(th_)
            _free_projs.append(fh_)

        with tc.tile_pool(name="res", bufs=1) as res:
            w_sb = res.tile([128, 768], f32)
            nc.sync.dma_start(out=w_sb, in_=W_d[:])
            b3T = res.tile([128, 8], f32)
            nc.sync.dma_start(out=b3T, in_=b3T_d[:])
            uzr = res.tile([128, 1024], bf16)
            nc.sync.dma_start(
                out=uzr[:].rearrange("p (i m) -> p i m", i=8),
                in_=uzr_d.rearrange("i p m -> p i m"),
            )
            uh = res.tile([128, 512], bf16)
            nc.sync.dma_start(
                out=uh[:].rearrange("p (i m) -> p i m", i=4),
                in_=uh_d.rearrange("i p m -> p i m"),
            )
            i128 = res.tile([128, 128], bf16)
            nc.sync.dma_start(out=i128, in_=i128_d[:])

            # ---- all pools coexist: prologue PSUM (2 banks) + recurrence
            # PSUM (6 banks) stay live together so phases can overlap ----
            with (
                tc.tile_pool(name="p1x", bufs=3) as p1x,
                tc.tile_pool(name="p1ps", bufs=1, space="PSUM") as p1ps,
                tc.tile_pool(name="p1o", bufs=3) as p1o,
                tc.tile_pool(name="p1z", bufs=1) as p1z,
                tc.tile_pool(name="projp", bufs=2) as projp,
                tc.tile_pool(name="projhp", bufs=2) as projhp,
                tc.tile_pool(name="decp", bufs=2) as decp,
                tc.tile_pool(name="hgp", bufs=2) as hgp,
                tc.tile_pool(name="wk", bufs=3) as wk,
                tc.tile_pool(name="pzr", bufs=2, space="PSUM") as pzrp,
                tc.tile_pool(name="phh", bufs=2, space="PSUM") as phhp,
            ):
                # prologue: proj = x @ W (+bias), transposed, to DRAM.
                # Supertile n covers t in [16n, 16n+16) = one segment s,
                # proj rows i0..i0+16 = groups i0//GS, i0//GS + 1.
                xs_tiles = {}

                def fetch_x(n):
                    xs = p1x.tile([128, 512], f32, tag="xs")
                    nc.sync.dma_start(out=xs, in_=xT_d[n])
                    xs_tiles[n] = xs

                # proj DRAM col layout is s-major: col = s*192 + gc*32 + b,
                # so each supertile stores one contiguous 192-col block with
                # just 2 DMAs (one per covered group)
                def emit_supertile(n):
                    if n not in xs_tiles:
                        fetch_x(n)
                    xs = xs_tiles.pop(n)
                    s, i0 = (16 * n) // L, (16 * n) % L + W
                    dup = s + 1 < S and (16 * n) % L == L - 16
                    psb = p1o.tile([128, 2048], bf16, tag="psb")
                    psh = p1o.tile([128, 1024], bf16, tag="psh")
                    for gc in range(6):
                        ps = p1ps.tile([128, 512], f32, tag=f"ps{gc % 2}")
                        nc.tensor.matmul(ps[:], w_sb[:, 128 * gc:128 * gc + 128],
                                         xs[:], start=True, stop=True)
                        if gc < 4:
                            # zr: bf16, col = ts16*128 + gc*32 + b
                            nc.scalar.activation(
                                out=psb[:].rearrange(
                                    "p (t r) -> p t r",
                                    t=16)[:, :, 32 * gc:32 * gc + 32],
                                in_=ps[:], func=IDN, bias=b3T[:, gc:gc + 1])
                        else:
                            # h: f32, col = ts16*64 + (gc-4)*32 + b
                            mc = gc - 4
                            nc.scalar.activation(
                                out=psh[:].rearrange(
                                    "p (t r) -> p t r",
                                    t=16)[:, :, 32 * mc:32 * mc + 32],
                                in_=ps[:], func=IDN, bias=b3T[:, gc:gc + 1])
                    for h_ in range(2):
                        g_ = i0 // GS + h_
                        nc.sync.dma_start(
                            out=projs[g_][:, :, 128 * s:128 * s + 128]
                            .rearrange("t p b -> p t b"),
                            in_=psb[:, 1024 * h_:1024 * h_ + 1024].rearrange(
                                "p (t b) -> p t b", b=128),
                        )
                        nc.sync.dma_start(
                            out=projhs[g_][:, :, 64 * s:64 * s + 64]
                            .rearrange("t p b -> p t b"),
                            in_=psh[:, 512 * h_:512 * h_ + 512].rearrange(
                                "p (t b) -> p t b", b=64),
                        )
                    if dup:
                        # last 8 steps also seed segment s+1 warmup (i'=ts16-8)
                        nc.sync.dma_start(
                            out=projs[0][:, :, 128 * (s + 1):128 * (s + 2)]
                            .rearrange("t p b -> p t b"),
                            in_=psb[:, 1024:2048].rearrange(
                                "p (t b) -> p t b", b=128),
                        )
                        nc.sync.dma_start(
                            out=projhs[0][:, :,
                                          64 * (s + 1):64 * (s + 1) + 64]
                            .rearrange("t p b -> p t b"),
                            in_=psh[:, 512:1024].rearrange(
                                "p (t b) -> p t b", b=64),
                        )

                # zero-fill segment-0 warmup rows: i<W
                zt = p1z.tile([128, W * 128], bf16)
                nc.any.memzero(zt)
                nc.sync.dma_start(
                    out=projs[0][:, :, 0:128].rearrange("t p b -> p t b"),
                    in_=zt[:].rearrange("p (t b) -> p t b", b=128),
                )
                zth = p1z.tile([128, W * 64], bf16)
                nc.any.memzero(zth)
                nc.sync.dma_start(
                    out=projhs[0][:, :, 0:64].rearrange("t p b -> p t b"),
                    in_=zth[:].rearrange("p (t b) -> p t b", b=64),
                )
                # n%4==3 supertiles write groups 0 (warmup dups), 7, 8:
                # emit before the loop; the rest interleave into the loop,
                # one supertile per step (writers stay ahead of consumers:
                # k-th quarter covers groups 2k+1, 2k+2, consumed from step
                # 8*(2k+1)).
                pre = [n for n in range(3, NSUP, 4)]
                inloop = [n for r in (0, 1, 2) for n in range(r, NSUP, 4)]
                if "noprol" in vset:
                    pre, inloop = [], []
                if "serprol" in vset or "norec" in vset:
                    pre, inloop = pre + inloop, []
                for n_ in pre[:3]:
                    fetch_x(n_)
                for j_, n in enumerate(pre):
                    if j_ + 3 < len(pre):
                        fetch_x(pre[j_ + 3])
                    emit_supertile(n)
                def load_proj(g):
                    t_ = projp.tile([128, GS * 1024], bf16, tag="pj")
                    nc.sync.dma_start(
                        out=t_[:].rearrange("p (i m) -> p i m", i=GS),
                        in_=projs[g][:].rearrange("i p m -> p i m"))
                    return t_

                def load_projh(g):
                    t_ = projhp.tile([128, GS * 512], bf16, tag="pjh")
                    nc.sync.dma_start(
                        out=t_[:].rearrange("p (i m) -> p i m", i=GS),
                        in_=projhs[g][:].rearrange("i p m -> p i m"))
                    return t_

                def preload_pr(i, pj_g):
                    t_ = pzrp.tile([128, 512], f32, tag="pr")
                    nc.tensor.matmul(
                        t_[:], i128[:],
                        pj_g[:, 1024 * (i % GS):1024 * (i % GS) + 1024]
                        .rearrange("p (s g b) -> p g s b", s=8, g=4)[:, 0:2],
                        start=True, stop=False)
                    return t_

                def preload_phh(i, pjh_g):
                    """Identity matmul copies proj_h into the h psum bank;
                    the step's Uh matmuls then accumulate on top."""
                    t_ = phhp.tile([128, 512], f32, tag="phh")
                    nc.tensor.matmul(
                        t_[:], i128[:],
                        pjh_g[:, 512 * (i % GS):512 * (i % GS) + 512]
                        .rearrange("p (s m b) -> p m s b", s=8, m=2),
                        start=True, stop=False)
                    return t_

                def load_dec(g):
                    t_ = decp.tile([128, GS * 512], bf16, tag="db")
                    nc.sync.dma_start(out=t_, in_=decb_d[g])
                    return t_

                # proj group g is loaded only after its prologue writers are
                # emitted: groups {2m-1, 2m} load at the end of step 8m-1
                # (their writers are the m-1'th quarter, emitted steps
                # 8(m-1)..8m-1; groups 0, 7, 8 are written by the pre-chunk)
                pj_tiles = {0: load_proj(0)}
                pjh_tiles = {0: load_projh(0)}
                decg, decg_nxt = load_dec(0), None
                hgrp = None
                phh = preload_phh(0, pjh_tiles[0])
                prr = preload_pr(0, pj_tiles[0])

                hdec = res.tile([128, 512], bf16)
                nc.any.memzero(hdec)  # hdec(0) = d(0)*0 = 0
                for n in inloop[:2]:
                    fetch_x(n)

                m_stop = M
                for v in vset:
                    if v.startswith("m") and v[1:].isdigit():
                        m_stop = int(v[1:])
                for i in range(0 if "norec" not in vset else M, m_stop):
                    g, il = i // GS, i % GS
                    projg = pj_tiles[g]
                    if il == 0:
                        if g + 1 < G:
                            decg_nxt = load_dec(g + 1)
                        hgrp = hgp.tile([128, GS * 512], bf16, tag="hh")
                    # proj cols are s-major (s*128 + gc*32 + b); psum cols are
                    # (gate, mc)-major -> strided 4-D views per gate
                    pj_step = projg[:, 1024 * il:1024 * il + 1024].rearrange(
                        "p (s g b) -> p g s b", s=8, g=4)
                    db_nxt = (decg[:, 512 * (il + 1):512 * (il + 2)]
                              if il + 1 < GS else
                              (decg_nxt[:, 0:512] if g + 1 < G else None))

                    pzz = pzrp.tile([128, 512], f32, tag="pz")
                    # r accumulates onto the preloaded proj_r fold; z opens
                    # its own psum group
                    for gate in range(2):
                        for mc in range(2):
                            for kc in range(2):
                                i0 = ((gate * 2 + mc) * 2 + kc) * 128
                                ps_g = prr if gate == 0 else pzz
                                nc.tensor.matmul(
                                    ps_g[:, mc * 256:mc * 256 + 256],
                                    uzr[:, i0:i0 + 128],
                                    hdec[:, 256 * kc:256 * kc + 256],
                                    start=(gate == 1 and kc == 0),
                                    stop=(kc == 1))

                    # r-path stays minimal on DVE/ACT; the z-path (pre_z,
                    # sigmoid-z's input, a) runs on the otherwise-idle
                    # gpsimd (Pool) engine, off the critical chain
                    ADD = mybir.AluOpType.add
                    zeng = nc.vector if "nogpz" in vset else nc.gpsimd
                    pre_z = wk.tile([128, 512], bf16, tag="prz")
                    nc.vector.tensor_tensor(out=pre_z, in0=pzz[:],
                                            in1=pj_step[:, 2:4, :, :], op=ADD)
                    # r-head split by mc halves: sigma/rh/Uh consume
                    # 256-col halves so the h-path starts earlier
                    fine = "nofine" not in vset
                    r_s = wk.tile([128, 512], bf16, tag="rs")
                    rh = wk.tile([128, 512], bf16, tag="rh")
                    if fine:
                        for mc in range(2):
                            sl = slice(256 * mc, 256 * mc + 256)
                            nc.scalar.activation(out=r_s[:, sl],
                                                 in_=prr[:, sl], func=SIG)
                            nc.vector.tensor_tensor(out=rh[:, sl],
                                                    in0=r_s[:, sl],
                                                    in1=hdec[:, sl], op=MUL)
                    else:
                        nc.scalar.activation(out=r_s, in_=prr[:], func=SIG)
                        nc.vector.tensor_tensor(out=rh, in0=r_s[:],
                                                in1=hdec[:], op=MUL)
                    z_s = wk.tile([128, 512], bf16, tag="zs")
                    nc.scalar.activation(out=z_s, in_=pre_z[:], func=SIG)
                    # a = (z-1)*hdec  (off critical path once z ready; gpsimd
                    # offload attempted but Pool TensorTensor wedges this
                    # runtime, so it stays on DVE)
                    a_t = wk.tile([128, 512], bf16, tag="at")
                    nc.vector.scalar_tensor_tensor(
                        out=a_t, in0=z_s[:], scalar=1.0, in1=hdec[:],
                        op0=SUB, op1=MUL)

                    # h-gate accumulates onto the preloaded proj_h psum;
                    # kc-major order so the kc0 pair only needs rh half 0
                    for kc in range(2):
                        for mc in range(2):
                            i0 = (mc * 2 + kc) * 128
                            nc.tensor.matmul(
                                phh[:, 256 * mc:256 * mc + 256],
                                uh[:, i0:i0 + 128],
                                rh[:, 256 * kc:256 * kc + 256],
                                start=False, stop=(kc == 1))

                    ht = wk.tile([128, 512], bf16, tag="ht")
                    nc.scalar.activation(out=ht, in_=phh[:], func=TANH)

                    bb = wk.tile([128, 512], bf16, tag="bb")
                    nc.vector.tensor_tensor(out=bb, in0=z_s[:], in1=ht[:],
                                            op=MUL)
                    h_new = hgrp[:, 512 * il:512 * il + 512]
                    nc.vector.tensor_tensor(out=h_new, in0=bb[:], in1=a_t[:],
                                            op=SUB)
                    if db_nxt is not None:
                        hdec_n = wk.tile([128, 512], bf16, tag="hd")
                        nc.vector.tensor_tensor(out=hdec_n, in0=h_new,
                                                in1=db_nxt, op=MUL)
                        hdec = hdec_n

                    if il == GS - 1:
                        if g >= 1:
                            nc.sync.dma_start(out=outG_d[g - 1], in_=hgrp[:])
                        decg = decg_nxt
                    # interleaved prologue: fill the end-of-step PE/ACT idle
                    # window; x-tile prefetched 2 steps ahead
                    if i + 2 < len(inloop):
                        fetch_x(inloop[i + 2])
                    if i < len(inloop):
                        emit_supertile(inloop[i])
                    if (i + 1) % GS == 0:
                        m_ = (i + 1) // GS
                        for gg in (2 * m_ - 1, 2 * m_):
                            if gg < G and gg not in pj_tiles:
                                pj_tiles[gg] = load_proj(gg)
                                pjh_tiles[gg] = load_projh(gg)
                        pj_tiles.pop(g, None)
                    # preload next step's h-gate psum via identity matmul
                    # (fills the end-of-step PE idle window)
                    if i + 1 < m_stop:
                        phh = preload_phh(i + 1, pjh_tiles[(i + 1) // GS])
                        prr = preload_pr(i + 1, pj_tiles[(i + 1) // GS])

        for f_ in _free_projs:
            f_()

    _split_matmul_waits(nc, mybir)
    return nc


def _split_matmul_waits(nc, mybir):
    """Walrus allows at most one sync wait per engine instruction. Move the
    excess onto same-engine NoOps inserted just before."""
    for func in nc.m.functions:
        for blk in func.blocks:
            new_insts = []
            for inst in blk.instructions:
                si = inst.sync_info
                if si is not None and len(si.on_wait) > 1:
                    extra = list(si.on_wait[:-1])
                    keep = [si.on_wait[-1]]
                    for w in extra:
                        nop = mybir.InstNoOp(
                            name=nc.get_next_instruction_name(),
                            sync_info=mybir.SyncInfo(on_wait=[w], on_update=[]),
                            engine=inst.engine,
                            bass_nofuse=True,
                        )
                        nc.register_instruction(nop)
                        new_insts.append(nop)
                    si.on_wait = keep
                new_insts.append(inst)
            blk.instructions[:] = new_insts


def _get_nc(variant=()):
    key = tuple(variant)
    if key not in _NC_CACHE:
        _NC_CACHE[key] = _build(variant)
    return _NC_CACHE[key]


def _prep_shared(Wr, Wz, Wh, Ur, Uz, Uh, br, bz, bh):
    import ml_dtypes
    bf = ml_dtypes.bfloat16
    Wr, Wz, Wh = (np.asarray(a, np.float32) for a in (Wr, Wz, Wh))
    Ur, Uz, Uh = (np.asarray(a, np.float32) for a in (Ur, Uz, Uh))
    br, bz, bh = (np.asarray(a, np.float32) for a in (br, bz, bh))
    # W_cat cols: gc*128 + m, gc = gate*2 + mc, gates (r, z, h)
    Wc = np.empty((128, 768), np.float32)
    b3T = np.zeros((128, 8), np.float32)
    for g, (Wg, bg) in enumerate(((Wr, br), (Wz, bz), (Wh, bh))):
        for mc in range(2):
            gc = g * 2 + mc
            Wc[:, 128 * gc:128 * gc + 128] = Wg[:, 128 * mc:128 * mc + 128]
            b3T[:, gc] = bg[128 * mc:128 * mc + 128]
    Uzr = np.empty((8, 128, 128), bf)
    for g, Ug in enumerate((Ur, Uz)):
        for mc in range(2):
            for kc in range(2):
                Uzr[(g * 2 + mc) * 2 + kc] = Ug[
                    128 * kc:128 * kc + 128, 128 * mc:128 * mc + 128].astype(bf)
    Uh4 = np.empty((4, 128, 128), bf)
    for mc in range(2):
        for kc in range(2):
            Uh4[mc * 2 + kc] = Uh[128 * kc:128 * kc + 128,
                                  128 * mc:128 * mc + 128].astype(bf)
    I128 = np.eye(128, dtype=np.float32).astype(bf)
    return dict(Wc=Wc, b3T=b3T, Uzr=Uzr, Uh4=Uh4, I128=I128)


def _prep_core(xs, ds):
    """xs [32, 512, 128] f32, ds [32, 512] f32 -> xT, decb."""
    import ml_dtypes
    bf = ml_dtypes.bfloat16
    xs = np.asarray(xs, np.float32)
    ds = np.asarray(ds, np.float32)
    # xT[n, d, 32*ts16 + b] = xs[b, 16n+ts16, d]
    xT = np.ascontiguousarray(
        xs.reshape(BS, NSUP, 16, 128).transpose(1, 3, 2, 0).reshape(
            NSUP, 128, 512))
    # d_pad[b, s*L + i] = (0 if i < W and s == 0 ... actually t = s*L+i-W)
    dpad = np.concatenate([np.zeros((BS, W), np.float32), ds], axis=1)
    # decb[g, p, il*512 + mc*256 + 32*s + b] = dpad[b, s*L + 8g + il]
    i_idx = np.arange(M)
    tp = (np.arange(S)[None, :] * L + i_idx[:, None])      # [M, S]
    dmi = dpad[:, tp]                                      # [b, M, S]
    dmi = dmi.transpose(1, 2, 0)                           # [M, S, b]
    dcol = np.concatenate([dmi, dmi], axis=1).reshape(M, 512)  # mc dup
    decb = np.ascontiguousarray(np.broadcast_to(
        dcol.reshape(G, 1, GS * 512), (G, 128, GS * 512)).astype(bf))
    return dict(xT=xT, decb=decb)


_EXEC_CACHE = {}


def _run_spmd(nc, in_maps, n_timed=0):
    """Multi-core exec via bass2jax/PJRT with optional wall timing."""
    import time
    import jax
    import jax.numpy as jnp
    from jax.sharding import Mesh, PartitionSpec
    from jax.experimental.shard_map import shard_map
    import concourse.mybir as mybir
    from concourse import bass2jax
    from concourse.bass2jax import _bass_exec_p, partition_id_tensor

    bass2jax.install_neuronx_cc_hook()
    if not nc.is_finalized():
        nc.finalize()
    if id(nc) in _EXEC_CACHE:
        return _EXEC_CACHE[id(nc)](in_maps, n_timed)

    partition_name = (nc.partition_id_tensor.name
                      if nc.partition_id_tensor else None)
    in_names, out_names, out_avals, zero_outs = [], [], [], []
    for alloc in nc.m.functions[0].allocations:
        if not isinstance(alloc, mybir.MemoryLocationSet):
            continue
        name = alloc.memorylocations[0].name
        if alloc.kind == "ExternalInput":
            if name != partition_name:
                in_names.append(name)
        elif alloc.kind == "ExternalOutput":
            aval = jax.core.ShapedArray(
                tuple(alloc.tensor_shape), mybir.dt.np(alloc.dtype))
            out_names.append(name)
            out_avals.append(aval)
            zero_outs.append(np.zeros(aval.shape, aval.dtype))

    n_params = len(in_names)
    all_names = list(in_names) + list(out_names)
    if partition_name is not None:
        all_names.append(partition_name)

    def _body(*args):
        operands = list(args)
        if partition_name is not None:
            operands.append(partition_id_tensor())
        return tuple(_bass_exec_p.bind(
            *operands,
            out_avals=tuple(out_avals),
            in_names=tuple(all_names),
            out_names=tuple(out_names),
            lowering_input_output_aliases=(),
            sim_require_finite=True,
            sim_require_nnan=True,
            nc=nc,
        ))

    devices = jax.devices()[:NCORES]
    mesh = Mesh(np.asarray(devices), ("core",))
    nio = n_params + len(out_names)
    sharded = jax.jit(shard_map(
        _body, mesh=mesh,
        in_specs=(PartitionSpec("core"),) * nio,
        out_specs=(PartitionSpec("core"),) * len(out_names),
        check_rep=False), keep_unused=True)

    def _runner(in_maps, n_timed):
        concat_in = [
            np.concatenate([np.asarray(m[name]) for m in in_maps], axis=0)
            for name in in_names]
        concat_zeros = [np.zeros((NCORES * z.shape[0], *z.shape[1:]), z.dtype)
                        for z in zero_outs]
        args = concat_in + concat_zeros

        out_arrs = sharded(*args)
        jax.block_until_ready(out_arrs)

        times = []
        if n_timed:
            sharding = jax.sharding.NamedSharding(mesh, PartitionSpec("core"))
            dev_args = [jax.device_put(a, sharding) for a in args]
            jax.block_until_ready(dev_args)

            def _timed(n):
                t0 = time.perf_counter()
                o = None
                for _ in range(n):
                    o = sharded(*dev_args)
                jax.block_until_ready(o)
                return time.perf_counter() - t0

            _timed(1)  # warm
            samples = []
            for _ in range(4):
                t1 = min(_timed(1) for _ in range(4))
                tn = _timed(1 + n_timed)
                samples.append((tn - t1) / n_timed)
            samples.sort()
            times = [samples[len(samples) // 2]]  # median estimate

        results = [
            {name: np.asarray(out_arrs[i]).reshape(
                NCORES, *out_avals[i].shape)[c]
             for i, name in enumerate(out_names)}
            for c in range(NCORES)
        ]
        return results, times

    _EXEC_CACHE[id(nc)] = _runner
    return _runner(in_maps, n_timed)


def _make_in_maps(x, h_decay, Wr, Wz, Wh, Ur, Uz, Uh, br, bz, bh):
    shared = _prep_shared(Wr, Wz, Wh, Ur, Uz, Uh, br, bz, bh)
    x = np.asarray(x, np.float32)
    h_decay = np.asarray(h_decay, np.float32)
    in_maps = []
    for c in range(NCORES):
        m = dict(shared)
        m.update(_prep_core(x[c * BS:(c + 1) * BS],
                            h_decay[c * BS:(c + 1) * BS]))
        in_maps.append(m)
    return in_maps


def _unshard_out(oG):
    """oG [G-1, 128, GS*512] bf16 -> [BS, T, H] f32.
    col = il*512 + mc*256 + 32*s + b; t = s*L + 8*(g+1) + il - W."""
    o = np.asarray(oG, np.float32).reshape(G - 1, 128, GS, 2, S, BS)
    # dims: (g, p, il, mc, s, b) -> (b, s, g, il, mc, p)
    o = o.transpose(5, 4, 0, 2, 3, 1).reshape(BS, S, (G - 1) * GS, H)
    return o.reshape(BS, T, H)


def kernel(x, h_decay, Wr, Wz, Wh, Ur, Uz, Uh, br, bz, bh):
    global LAST_EXEC_NS
    nc = _get_nc()
    in_maps = _make_in_maps(x, h_decay, Wr, Wz, Wh, Ur, Uz, Uh, br, bz, bh)
    n_timed = 5 if TRACE else 0
    results, times = _run_spmd(nc, in_maps, n_timed=n_timed)
    if times:
        LAST_EXEC_NS = int(min(times) * 1e9)

    out = np.empty((B, T, H), np.float32)
    for c in range(NCORES):
        out[c * BS:(c + 1) * BS] = _unshard_out(results[c]["outG"])
    return out



# revision 7
# speedup vs baseline: 1.8232x; 1.8232x over previous
"""GRU-D layer kernel v3: JIT input projections, no DRAM staging.

Shapes: x [256, 512, 128], h_decay [256, 512], H=256. 8 cores, batch 32/core.

Sequence split: S=8 segments of L=64 steps, W=8 warmup steps (contraction
(1-z)*d makes truncation error ~1e-4; bf16 rounding dominates).
Per core: V=256 virtual sequences (32 batch x 8 segments), M=72 macro-steps.

v3 design (vs v2): the input projections x@W are computed just-in-time on
the PE straight into the step's PSUM banks (start=True), so there is no
DRAM proj scratch, no f32 prologue matmuls, no identity preload matmuls,
and no IDENTITY bias pass on ACT (bias rides the sigmoid/tanh activation's
per-partition bias port). Decay factors stay in SBUF (bf16) so the state
multiply runs in the DVE 2x packed mode. Dense PE work (next step's proj
seeds issued each step) keeps the HAM clock gate at 2.4 GHz.

Layouts (per core):
  h state      [128 p, 512]  bf16, col = mc*256 + 32*s + b  (h = 128*mc + p)
  psum r/z/h   [128, 512] f32 each, col = mc*256 + v, double-buffered
  xsg  DRAM    [128, S*M*32] bf16, col = (s*M + i)*32 + b, = x[b, s*64+i-8, d]
               (zeros for s=0, i<W)
  decb DRAM    [128, M*512] bf16 (d broadcast, col = i*512 + mc*256 + 32s+b)
  out          [8, 128, 8*512] bf16 groups (i>=8), host converts to f32

Recurrence per macro-step:
  PE : 4 mm r-gate, 4 mm z-gate (on hdec), 4 mm h-gate (on rh),
       then 6 proj mm seeding step i+1's psum banks
  ACT: r = sig(psum_r + br); z = sig(psum_z + bz); ht = tanh(psum_h + bh)
       (per-mc halves so bias is per-partition and halves pipeline)
  DVE: rh = r*hdec; per mc: d = hdec - ht; e = z*d; hnew = hdec - e;
       hdec' = hnew * dec   (h_t = (1-z)*hdec + z*ht = hdec - z*(hdec-ht))
"""

import numpy as np

B, T, D, H = 256, 512, 128, 256
NCORES = 8
BS = B // NCORES          # 32 batch per core
S = 8                     # segments
L = T // S                # 64
W = 8                     # warmup steps
M = L + W                 # 72 macro-steps
V = BS * S                # 256 virtual seqs per core
GS = 8                    # steps per out group
G = M // GS               # 9 groups (group 0 = warmup, no output)

TRACE = False
LAST_EXEC_NS = None

_NC_CACHE = {}


def _build(variant=()):
    vset = set(variant)
    import concourse.bass as bass
    import concourse.mybir as mybir
    from concourse.tile import TileContext

    f32 = mybir.dt.float32
    bf16 = mybir.dt.bfloat16
    SIG = mybir.ActivationFunctionType.Sigmoid
    TANH = mybir.ActivationFunctionType.Tanh
    MUL = mybir.AluOpType.mult
    SUB = mybir.AluOpType.subtract

    nc = bass.Bass()
    xs_d = nc.dram_tensor("xsg", [128, S * M * BS], f32, kind="ExternalInput")
    W_d = nc.dram_tensor("Wc", [128, 768], f32, kind="ExternalInput")
    b3T_d = nc.dram_tensor("b3T", [128, 8], f32, kind="ExternalInput")
    uzr_d = nc.dram_tensor("Uzr", [8, 128, 128], bf16, kind="ExternalInput")
    uh_d = nc.dram_tensor("Uh4", [4, 128, 128], bf16, kind="ExternalInput")
    decb_d = nc.dram_tensor("decb", [128, M * 512], bf16,
                            kind="ExternalInput")
    outG_d = nc.dram_tensor("outG", [G - 1, 128, GS * 512], bf16,
                            kind="ExternalOutput")

    with TileContext(nc) as tc:
        with tc.tile_pool(name="res", bufs=1) as res:
            w_sb = res.tile([128, 768], f32)
            nc.sync.dma_start(out=w_sb, in_=W_d[:])
            b3T = res.tile([128, 8], f32)
            nc.sync.dma_start(out=b3T, in_=b3T_d[:])
            uzr = res.tile([128, 1024], bf16)
            nc.sync.dma_start(
                out=uzr[:].rearrange("p (i m) -> p i m", i=8),
                in_=uzr_d.rearrange("i p m -> p i m"),
            )
            uh = res.tile([128, 512], bf16)
            nc.sync.dma_start(
                out=uh[:].rearrange("p (i m) -> p i m", i=4),
                in_=uh_d.rearrange("i p m -> p i m"),
            )
            # x resident in SBUF, split DMAs (one per segment) to spread
            # across queues; step 0 only needs each segment's first slice
            xs = res.tile([128, S * M * BS], f32)
            for s_ in range(S):
                sl = slice(s_ * M * BS, (s_ + 1) * M * BS)
                nc.sync.dma_start(out=xs[:, sl], in_=xs_d[:, sl])
            # decay factors resident in SBUF, split by group
            dec = res.tile([128, M * 512], bf16)
            for g_ in range(G):
                sl = slice(g_ * GS * 512, (g_ + 1) * GS * 512)
                nc.sync.dma_start(out=dec[:, sl], in_=decb_d[:, sl])

            xv = xs[:].rearrange("p (s i b) -> p i s b", s=S, i=M)

            with (
                tc.tile_pool(name="wk", bufs=3) as wk,
                tc.tile_pool(name="hg", bufs=2) as hgp,
                tc.tile_pool(name="ps", bufs=2, space="PSUM") as psp,
            ):
                def proj_step(j):
                    """Seed step j's psum banks with W@x_j (start=True)."""
                    pr = psp.tile([128, 512], f32, tag="pr")
                    pz = psp.tile([128, 512], f32, tag="pz")
                    ph = psp.tile([128, 512], f32, tag="ph")
                    # mc0 opens the bank's accumulation group (whole 2KB
                    # zero region goes pending-zero); mc1 overwrites its
                    # pending bytes. The group closes at the last U matmul.
                    for gate, pt in ((0, pr), (1, pz), (2, ph)):
                        for mc in range(2):
                            gc = gate * 2 + mc
                            nc.tensor.matmul(
                                pt[:, 256 * mc:256 * mc + 256],
                                w_sb[:, 128 * gc:128 * gc + 128],
                                xv[:, j:j + 1], start=(mc == 0), stop=False)
                    return pr, pz, ph

                hdec = wk.tile([128, 512], bf16, tag="hd")
                nc.any.memzero(hdec)
                pr, pz, ph = proj_step(0)
                hgrp = None

                gpl = 1 if "gp" in vset else 0
                for i in range(M):
                    g, il = i // GS, i % GS
                    if il == 0:
                        hgrp = hgp.tile([128, GS * 512], bf16, tag="hh")

                    # r-gate: mc-major so sig_r(mc0) starts after 2 mm
                    for mc in range(2):
                        for kc in range(2):
                            uidx = (0 * 2 + mc) * 2 + kc
                            nc.tensor.matmul(
                                pr[:, 256 * mc:256 * mc + 256],
                                uzr[:, 128 * uidx:128 * uidx + 128],
                                hdec[:, 256 * kc:256 * kc + 256],
                                start=False, stop=(mc == 1 and kc == 1))
                    # z-gate
                    for mc in range(2):
                        for kc in range(2):
                            uidx = (1 * 2 + mc) * 2 + kc
                            nc.tensor.matmul(
                                pz[:, 256 * mc:256 * mc + 256],
                                uzr[:, 128 * uidx:128 * uidx + 128],
                                hdec[:, 256 * kc:256 * kc + 256],
                                start=False, stop=(mc == 1 and kc == 1))

                    r_s = wk.tile([128, 512], bf16, tag="rs")
                    z_s = wk.tile([128, 512], bf16, tag="zs")
                    ht = wk.tile([128, 512], bf16, tag="ht")
                    rh = wk.tile([128, 512], bf16, tag="rh")
                    for mc in range(2):
                        sl = slice(256 * mc, 256 * mc + 256)
                        nc.scalar.activation(out=r_s[:, sl], in_=pr[:, sl],
                                             func=SIG, bias=b3T[:, mc:mc + 1])
                        nc.vector.tensor_tensor(out=rh[:, sl], in0=r_s[:, sl],
                                                in1=hdec[:, sl], op=MUL)

                    # h-gate: kc-major, kc pair only needs rh half kc
                    for kc in range(2):
                        for mc in range(2):
                            uidx = mc * 2 + kc
                            nc.tensor.matmul(
                                ph[:, 256 * mc:256 * mc + 256],
                                uh[:, 128 * uidx:128 * uidx + 128],
                                rh[:, 256 * kc:256 * kc + 256],
                                start=False, stop=(kc == 1 and mc == 1))

                    # seed next step's psum while ACT/DVE run the tail
                    prn = pzn = phn = None
                    if i + 1 < M:
                        prn, pzn, phn = proj_step(i + 1)

                    nc.scalar.activation(out=z_s[:, 0:256], in_=pz[:, 0:256],
                                         func=SIG, bias=b3T[:, 2:3])
                    nc.scalar.activation(out=ht[:, 0:256], in_=ph[:, 0:256],
                                         func=TANH, bias=b3T[:, 4:5])
                    nc.scalar.activation(out=z_s[:, 256:512],
                                         in_=pz[:, 256:512],
                                         func=SIG, bias=b3T[:, 3:4])
                    nc.scalar.activation(out=ht[:, 256:512],
                                         in_=ph[:, 256:512],
                                         func=TANH, bias=b3T[:, 5:6])

                    # tail: h_t = hdec - z*(hdec - ht); hdec' = h_t * dec_next
                    hdec_n = wk.tile([128, 512], bf16, tag="hd")
                    d_t = wk.tile([128, 512], bf16, tag="dt")
                    e_t = wk.tile([128, 512], bf16, tag="et")
                    for mc in range(2):
                        sl = slice(256 * mc, 256 * mc + 256)
                        eng = nc.gpsimd if (gpl and mc == 1) else nc.vector
                        h_new = hgrp[:, 512 * il + 256 * mc:
                                     512 * il + 256 * mc + 256]
                        eng.tensor_tensor(out=d_t[:, sl], in0=hdec[:, sl],
                                          in1=ht[:, sl], op=SUB)
                        eng.tensor_tensor(out=e_t[:, sl], in0=z_s[:, sl],
                                          in1=d_t[:, sl], op=MUL)
                        eng.tensor_tensor(out=h_new, in0=hdec[:, sl],
                                          in1=e_t[:, sl], op=SUB)
                        if i + 1 < M:
                            dsl = slice(512 * (i + 1) + 256 * mc,
                                        512 * (i + 1) + 256 * mc + 256)
                            eng.tensor_tensor(out=hdec_n[:, sl], in0=h_new,
                                              in1=dec[:, dsl], op=MUL)
                    hdec = hdec_n
                    pr, pz, ph = prn, pzn, phn

                    if il == GS - 1 and g >= 1:
                        nc.sync.dma_start(out=outG_d[g - 1], in_=hgrp[:])

    _split_matmul_waits(nc, mybir)
    return nc


def _split_matmul_waits(nc, mybir):
    """Walrus allows at most one sync wait per engine instruction. Move the
    excess onto same-engine NoOps inserted just before."""
    for func in nc.m.functions:
        for blk in func.blocks:
            new_insts = []
            for inst in blk.instructions:
                si = inst.sync_info
                if si is not None and len(si.on_wait) > 1:
                    extra = list(si.on_wait[:-1])
                    keep = [si.on_wait[-1]]
                    for w in extra:
                        nop = mybir.InstNoOp(
                            name=nc.get_next_instruction_name(),
                            sync_info=mybir.SyncInfo(on_wait=[w], on_update=[]),
                            engine=inst.engine,
                            bass_nofuse=True,
                        )
                        nc.register_instruction(nop)
                        new_insts.append(nop)
                    si.on_wait = keep
                new_insts.append(inst)
            blk.instructions[:] = new_insts


def _get_nc(variant=()):
    key = tuple(variant)
    if key not in _NC_CACHE:
        _NC_CACHE[key] = _build(variant)
    return _NC_CACHE[key]


def _prep_shared(Wr, Wz, Wh, Ur, Uz, Uh, br, bz, bh):
    import ml_dtypes
    bf = ml_dtypes.bfloat16
    Wr, Wz, Wh = (np.asarray(a, np.float32) for a in (Wr, Wz, Wh))
    Ur, Uz, Uh = (np.asarray(a, np.float32) for a in (Ur, Uz, Uh))
    br, bz, bh = (np.asarray(a, np.float32) for a in (br, bz, bh))
    # W_cat cols: gc*128 + m, gc = gate*2 + mc, gates (r, z, h)
    Wc = np.empty((128, 768), np.float32)
    b3T = np.zeros((128, 8), np.float32)
    for g, (Wg, bg) in enumerate(((Wr, br), (Wz, bz), (Wh, bh))):
        for mc in range(2):
            gc = g * 2 + mc
            Wc[:, 128 * gc:128 * gc + 128] = Wg[:, 128 * mc:128 * mc + 128]
            b3T[:, gc] = bg[128 * mc:128 * mc + 128]
    Uzr = np.empty((8, 128, 128), bf)
    for g, Ug in enumerate((Ur, Uz)):
        for mc in range(2):
            for kc in range(2):
                Uzr[(g * 2 + mc) * 2 + kc] = Ug[
                    128 * kc:128 * kc + 128, 128 * mc:128 * mc + 128].astype(bf)
    Uh4 = np.empty((4, 128, 128), bf)
    for mc in range(2):
        for kc in range(2):
            Uh4[mc * 2 + kc] = Uh[128 * kc:128 * kc + 128,
                                  128 * mc:128 * mc + 128].astype(bf)
    return dict(Wc=Wc, b3T=b3T, Uzr=Uzr, Uh4=Uh4)


def _prep_core(xs, ds):
    """xs [32, 512, 128] f32, ds [32, 512] f32 -> xsg, decb."""
    import ml_dtypes
    bf = ml_dtypes.bfloat16
    xs = np.asarray(xs, np.float32)
    ds = np.asarray(ds, np.float32)
    # xsg[d, (s*M + i)*32 + b] = xpad[b, s*64 + i, d], xpad t' = t + W
    xpad = np.concatenate([np.zeros((BS, W, D), np.float32), xs], axis=1)
    tg = (np.arange(S)[:, None] * L + np.arange(M)[None, :])  # [S, M]
    xg = xpad[:, tg, :]                                       # [b, S, M, d]
    xsg = np.ascontiguousarray(
        xg.transpose(3, 1, 2, 0).reshape(128, S * M * BS))
    # decb[p, i*512 + mc*256 + 32*s + b] = dpad[b, s*L + i]
    dpad = np.concatenate([np.zeros((BS, W), np.float32), ds], axis=1)
    # hdec entering t=0 is d_0 * h_init = 0 in the reference; zeroing this
    # (uniquely-indexed) entry keeps segment-0's warmup bias residue from
    # leaking into the real steps.
    dpad[:, W] = 0.0
    tp = (np.arange(S)[None, :] * L + np.arange(M)[:, None])   # [M, S]
    dmi = dpad[:, tp].transpose(1, 2, 0)                       # [M, S, b]
    dcol = np.concatenate([dmi, dmi], axis=1).reshape(M * 512)
    decb = np.ascontiguousarray(np.broadcast_to(
        dcol[None, :], (128, M * 512)).astype(bf))
    return dict(xsg=xsg, decb=decb)


_EXEC_CACHE = {}


def _run_spmd(nc, in_maps, n_timed=0):
    """Multi-core exec via bass2jax/PJRT with optional wall timing."""
    import time
    import jax
    import jax.numpy as jnp
    from jax.sharding import Mesh, PartitionSpec
    from jax.experimental.shard_map import shard_map
    import concourse.mybir as mybir
    from concourse import bass2jax
    from concourse.bass2jax import _bass_exec_p, partition_id_tensor

    bass2jax.install_neuronx_cc_hook()
    if not nc.is_finalized():
        nc.finalize()
    if id(nc) in _EXEC_CACHE:
        return _EXEC_CACHE[id(nc)](in_maps, n_timed)

    partition_name = (nc.partition_id_tensor.name
                      if nc.partition_id_tensor else None)
    in_names, out_names, out_avals, zero_outs = [], [], [], []
    for alloc in nc.m.functions[0].allocations:
        if not isinstance(alloc, mybir.MemoryLocationSet):
            continue
        name = alloc.memorylocations[0].name
        if alloc.kind == "ExternalInput":
            if name != partition_name:
                in_names.append(name)
        elif alloc.kind == "ExternalOutput":
            aval = jax.core.ShapedArray(
                tuple(alloc.tensor_shape), mybir.dt.np(alloc.dtype))
            out_names.append(name)
            out_avals.append(aval)
            zero_outs.append(np.zeros(aval.shape, aval.dtype))

    n_params = len(in_names)
    all_names = list(in_names) + list(out_names)
    if partition_name is not None:
        all_names.append(partition_name)

    def _body(*args):
        operands = list(args)
        if partition_name is not None:
            operands.append(partition_id_tensor())
        return tuple(_bass_exec_p.bind(
            *operands,
            out_avals=tuple(out_avals),
            in_names=tuple(all_names),
            out_names=tuple(out_names),
            lowering_input_output_aliases=(),
            sim_require_finite=True,
            sim_require_nnan=True,
            nc=nc,
        ))

    devices = jax.devices()[:NCORES]
    mesh = Mesh(np.asarray(devices), ("core",))
    nio = n_params + len(out_names)
    sharded = jax.jit(shard_map(
        _body, mesh=mesh,
        in_specs=(PartitionSpec("core"),) * nio,
        out_specs=(PartitionSpec("core"),) * len(out_names),
        check_rep=False), keep_unused=True)

    def _runner(in_maps, n_timed):
        concat_in = [
            np.concatenate([np.asarray(m[name]) for m in in_maps], axis=0)
            for name in in_names]
        concat_zeros = [np.zeros((NCORES * z.shape[0], *z.shape[1:]), z.dtype)
                        for z in zero_outs]
        args = concat_in + concat_zeros

        out_arrs = sharded(*args)
        jax.block_until_ready(out_arrs)

        times = []
        if n_timed:
            sharding = jax.sharding.NamedSharding(mesh, PartitionSpec("core"))
            dev_args = [jax.device_put(a, sharding) for a in args]
            jax.block_until_ready(dev_args)

            def _timed(n):
                t0 = time.perf_counter()
                o = None
                for _ in range(n):
                    o = sharded(*dev_args)
                jax.block_until_ready(o)
                return time.perf_counter() - t0

            _timed(1)  # warm
            samples = []
            for _ in range(4):
                t1 = min(_timed(1) for _ in range(4))
                tn = _timed(1 + n_timed)
                samples.append((tn - t1) / n_timed)
            samples.sort()
            times = [samples[len(samples) // 2]]  # median estimate

        results = [
            {name: np.asarray(out_arrs[i]).reshape(
                NCORES, *out_avals[i].shape)[c]
             for i, name in enumerate(out_names)}
            for c in range(NCORES)
        ]
        return results, times

    _EXEC_CACHE[id(nc)] = _runner
    return _runner(in_maps, n_timed)


def _make_in_maps(x, h_decay, Wr, Wz, Wh, Ur, Uz, Uh, br, bz, bh):
    shared = _prep_shared(Wr, Wz, Wh, Ur, Uz, Uh, br, bz, bh)
    x = np.asarray(x, np.float32)
    h_decay = np.asarray(h_decay, np.float32)
    in_maps = []
    for c in range(NCORES):
        m = dict(shared)
        m.update(_prep_core(x[c * BS:(c + 1) * BS],
                            h_decay[c * BS:(c + 1) * BS]))
        in_maps.append(m)
    return in_maps


def _unshard_out(oG):
    """oG [G-1, 128, GS*512] bf16 -> [BS, T, H] f32.
    col = il*512 + mc*256 + 32*s + b; t = s*L + 8*(g+1) + il - W."""
    o = np.asarray(oG, np.float32).reshape(G - 1, 128, GS, 2, S, BS)
    # dims: (g, p, il, mc, s, b) -> (b, s, g, il, mc, p)
    o = o.transpose(5, 4, 0, 2, 3, 1).reshape(BS, S, (G - 1) * GS, H)
    return o.reshape(BS, T, H)


def kernel(x, h_decay, Wr, Wz, Wh, Ur, Uz, Uh, br, bz, bh):
    global LAST_EXEC_NS
    nc = _get_nc()
    in_maps = _make_in_maps(x, h_decay, Wr, Wz, Wh, Ur, Uz, Uh, br, bz, bh)
    n_timed = 5 if TRACE else 0
    results, times = _run_spmd(nc, in_maps, n_timed=n_timed)
    if times:
        LAST_EXEC_NS = int(min(times) * 1e9)

    out = np.empty((B, T, H), np.float32)
    for c in range(NCORES):
        out[c * BS:(c + 1) * BS] = _unshard_out(results[c]["outG"])
    return out
